# revision 10
# baseline (speedup 1.0000x reference)
"""Trainium2 Bass kernel: multi-head encoder-decoder attention.

nn_MultiHeadEncDecAttention — B=1, N=4096, d_model=768, 12 heads, d_k=64.

Self-contained harness entry point: `kernel(**inputs) -> np.ndarray` takes
the FULL unsharded inputs (as produced by the problem's setup_inputs()),
distributes the work across 8 NeuronCores (heads x query-range sharding,
see below), runs a Bass/Tile SPMD program, and reassembles the full output.

Sharding: core pair p in {0..3} owns heads {3p, 3p+1, 3p+2}; within a
pair, core 2p handles query rows [0, 2048) and core 2p+1 rows [2048, 4096).
Each core computes its heads' attention for its query rows plus the partial
output projection for those heads; the host sums the 4 head-partials per
query half (b_o and the b_v contribution are folded in on the host, which
is exact because softmax rows sum to 1).
"""

import sys

sys.path.insert(0, "/opt/trn_rl_repo")

from contextlib import ExitStack

import numpy as np
import ml_dtypes

import concourse.tile as tile
from concourse import bacc, mybir
from concourse.bass_utils import run_bass_kernel_spmd

F32 = mybir.dt.float32
F32R = mybir.dt.float32r
BF16 = mybir.dt.bfloat16

D = 768          # d_model
DK = 64          # per-head dim
HPC = 3          # heads per core
P = 128          # SBUF partitions
QB = 512         # matmul moving-dim block
DT = D // P      # contraction k-tiles over d_model
N_CORES = 8


def build_program(NQ=2048, NK=4096, score_dt=BF16, kgroup=2):
    """Build + compile the per-core SPMD program (identical on all cores).

    Layout strategy (everything "transposed"; host passes x^T / enc^T):
      QT[h] [64, NQ], KT[h] [64, NK]  — duplicated to both partition halves
                                        so scores matmuls can PE-row-tile
      V[h]  [NK, 65]                  — natural layout via bf16 DMA
                                        transpose; column 64 = ones
      scoresT = KT-tile.T @ QT-block  -> PSUM [128(kpos), 512(q)]
      expT    = exp(0.125*scoresT)    -> SBUF bf16 (ACT, scale folded in;
                                        no max-subtraction needed: |s/8|<~6)
      AV      = [V|1].T @ expT        -> PSUM [65, 512]; row 64 = denom
      yT     += w_o-slice.T @ (AV[0:64] * recip(AV[64]))  over heads
    """
    KT_N = NK // P           # kpos tiles
    QBS = NQ // QB           # q blocks
    KB_N = NK // QB          # kpos blocks for the K/V projection
    EXP_TILES = 12 if KT_N >= 12 else KT_N   # kpos-tiles per exp buffer


    nc = bacc.Bacc("TRN2", target_bir_lowering=False, debug=False)

    xT = nc.dram_tensor("xT", [D, NQ], F32R, kind="ExternalInput").ap()
    encT = nc.dram_tensor("encT", [D, NK], F32R, kind="ExternalInput").ap()
    # wkv = concat([w_k cols, w_v cols]); projection passes use 128-col
    # slices: [wk0|wk1], [wk2|wv0], [wv1|wv2]
    wkv = nc.dram_tensor("wkv", [D, 2 * HPC * DK], F32R, kind="ExternalInput").ap()
    wq = nc.dram_tensor("wq", [D, HPC * DK], F32R, kind="ExternalInput").ap()
    wo = nc.dram_tensor("wo", [HPC * DK, D], F32R, kind="ExternalInput").ap()
    bq = nc.dram_tensor("bq", [HPC * DK, 1], F32, kind="ExternalInput").ap()
    bk = nc.dram_tensor("bk", [HPC * DK, 1], F32, kind="ExternalInput").ap()
    onesk = nc.dram_tensor("onesk", [1, NK // P], BF16, kind="ExternalInput").ap()
    ones64 = nc.dram_tensor("ones64", [1, DK], F32R, kind="ExternalInput").ap()
    yT = nc.dram_tensor("yT", [D, NQ], F32, kind="ExternalOutput").ap()

    with tile.TileContext(nc) as tc, ExitStack() as ctx:
        consts = ctx.enter_context(tc.tile_pool(name="consts", bufs=1))
        persist = ctx.enter_context(tc.tile_pool(name="persist", bufs=1))
        stream = ctx.enter_context(tc.tile_pool(name="stream", bufs=2))
        small = ctx.enter_context(tc.tile_pool(name="small", bufs=2))
        ysb_pool = ctx.enter_context(tc.tile_pool(name="ysb", bufs=1))
        exp_pool = ctx.enter_context(tc.tile_pool(name="exp", bufs=4))
        ps_s = ctx.enter_context(tc.tile_pool(name="ps_s", bufs=2, space="PSUM"))
        ps_mm = ctx.enter_context(tc.tile_pool(name="ps_mm", bufs=2, space="PSUM"))

        # ---- constants -------------------------------------------------
        wkv_sb = consts.tile([P, DT, 2 * HPC * DK], F32R)
        nc.sync.dma_start(out=wkv_sb, in_=wkv.rearrange("(t p) c -> p t c", p=P))
        wq_sb = consts.tile([P, DT, HPC * DK], F32R)
        nc.sync.dma_start(out=wq_sb, in_=wq.rearrange("(t p) c -> p t c", p=P))
        wo_sb = consts.tile([DK, HPC, D], F32R)
        nc.sync.dma_start(out=wo_sb, in_=wo.rearrange("(h d) n -> d h n", d=DK))
        bqA = consts.tile([P, 1], F32)
        nc.sync.dma_start(out=bqA, in_=bq[0:P, :])
        bqB = consts.tile([DK, 1], F32)
        nc.sync.dma_start(out=bqB, in_=bq[P : P + DK, :])
        bkA = consts.tile([P, 1], F32)
        nc.sync.dma_start(out=bkA, in_=bk[0:P, :])
        bkB = consts.tile([DK, 1], F32)
        nc.sync.dma_start(out=bkB, in_=bk[P : P + DK, :])
        ones1 = consts.tile([P, DK], F32R)
        nc.sync.dma_start(out=ones1[DK : DK + 1, :], in_=ones64)

        # ---- persistent per-head tensors ------------------------------
        kT = [persist.tile([P, NK], score_dt, name=f"kT{h}") for h in range(HPC)]
        qT = [persist.tile([P, NQ], score_dt, name=f"qT{h}") for h in range(HPC)]
        # V row stride padded to 80 els (160 B) so each DMA-transpose dest is
        # 32-byte aligned (XBAR requirement); col 64 = ones, 65..79 unused
        v = [persist.tile([P, KT_N, 80], BF16, name=f"v{h}") for h in range(HPC)]
        # vt shares the exp pool tag: dead after the V transposes, so its
        # slots are recycled as exp buffers during attention
        vt = [exp_pool.tile([P, NK], BF16, name=f"vt{h}", tag="e") for h in range(HPC)]

        # ---- K/V projection -------------------------------------------
        for kb in range(KB_N):
            ks = slice(kb * QB, (kb + 1) * QB)
            enc_t = stream.tile([P, DT, QB], F32R, name="enc_t", tag="enc")
            nc.sync.dma_start(
                out=enc_t, in_=encT.rearrange("(t p) n -> p t n", p=P)[:, :, ks]
            )
            for pi in range(3):
                ps = ps_mm.tile([P, QB], F32, tag="mm", name="ps_kv")
                for t in range(DT):
                    nc.tensor.matmul(
                        ps,
                        wkv_sb[:, t, pi * P : (pi + 1) * P],
                        enc_t[:, t, :],
                        start=(t == 0),
                        stop=(t == DT - 1),
                    )
                if pi == 0:
                    nc.vector.tensor_scalar_add(
                        out=kT[0][0:DK, ks], in0=ps[0:DK], scalar1=bkA[0:DK]
                    )
                    nc.vector.tensor_scalar_add(
                        out=kT[1][DK:P, ks], in0=ps[DK:P], scalar1=bkA[DK:P]
                    )
                elif pi == 1:
                    nc.vector.tensor_scalar_add(
                        out=kT[2][0:DK, ks], in0=ps[0:DK], scalar1=bkB[0:DK]
                    )
                    nc.vector.tensor_copy(out=vt[0][DK:P, ks], in_=ps[DK:P])
                else:
                    nc.vector.tensor_copy(out=vt[1][0:DK, ks], in_=ps[0:DK])
                    nc.vector.tensor_copy(out=vt[2][DK:P, ks], in_=ps[DK:P])

        # duplicate each head's K^T into the other partition half (the
        # scores matmuls PE-row-tile, so both operands must be resident
        # on both halves)
        nc.sync.dma_start(out=kT[0][DK:P, :], in_=kT[0][0:DK, :])
        nc.sync.dma_start(out=kT[1][0:DK, :], in_=kT[1][DK:P, :])
        nc.sync.dma_start(out=kT[2][DK:P, :], in_=kT[2][0:DK, :])

        # ---- V: bf16 DMA transpose into natural layout + ones column ---
        # (emitted before the Q projection so these DMAs don't queue
        # behind the xT stream — AV of the first unit needs V early)
        for h in range(HPC):
            src_rows = slice(DK, P) if h != 1 else slice(0, DK)
            nc.sync.dma_start(
                out=v[h][:, :, 0:DK], in_=vt[h][src_rows, :], transpose=True
            )
            nc.sync.dma_start(
                out=v[h][:, :, DK : DK + 1],
                in_=onesk.to_broadcast([P, KT_N]),
            )

        # ---- Q projection (per-block, with per-block half dup) ---------
        for qb in range(QBS):
            qs = slice(qb * QB, (qb + 1) * QB)
            x_t = stream.tile([P, DT, QB], F32R, name="x_t", tag="enc")
            nc.sync.dma_start(
                out=x_t, in_=xT.rearrange("(t p) n -> p t n", p=P)[:, :, qs]
            )
            ps = ps_mm.tile([P, QB], F32, tag="mm", name="ps_q01")
            for t in range(DT):
                nc.tensor.matmul(
                    ps,
                    wq_sb[:, t, 0:P],
                    x_t[:, t, :],
                    start=(t == 0),
                    stop=(t == DT - 1),
                )
            nc.vector.tensor_scalar_add(
                out=qT[0][0:DK, qs], in0=ps[0:DK], scalar1=bqA[0:DK]
            )
            nc.vector.tensor_scalar_add(
                out=qT[1][DK:P, qs], in0=ps[DK:P], scalar1=bqA[DK:P]
            )
            ps2 = ps_mm.tile([P, QB], F32, tag="mm", name="ps_q2")
            for t in range(DT):
                nc.tensor.matmul(
                    ps2[0:DK],
                    wq_sb[:, t, P : P + DK],
                    x_t[:, t, :],
                    start=(t == 0),
                    stop=(t == DT - 1),
                )
            nc.vector.tensor_scalar_add(
                out=qT[2][0:DK, qs], in0=ps2[0:DK], scalar1=bqB[0:DK]
            )
            nc.sync.dma_start(out=qT[0][DK:P, qs], in_=qT[0][0:DK, qs])
            nc.sync.dma_start(out=qT[1][0:DK, qs], in_=qT[1][DK:P, qs])
            nc.sync.dma_start(out=qT[2][DK:P, qs], in_=qT[2][0:DK, qs])

        # ---- attention + output projection ----------------------------
        # Software pipeline across (qb, h) units: emit scores+exp for unit
        # i, then AV+normalize for unit i-1 (whose exp overlapped unit i's
        # scores on ACT), then the output projection for a q block once its
        # last head is normalized.  Keeps the in-order PE stream free of
        # head-of-line waits on ACT.
        NGRP = (KT_N + kgroup - 1) // kgroup
        units = [(qb, h) for qb in range(QBS) for h in range(HPC)]
        oT = {}

        def emit_scores_exp(qb, h):
            qs = slice(qb * QB, (qb + 1) * QB)
            exp_bufs = []
            for g in range(NGRP):
                gsz = min(kgroup, KT_N - g * kgroup)
                ps = ps_s.tile([P, kgroup * QB], F32, tag="s", name="ps_sc")
                for j in range(gsz):
                    kt = g * kgroup + j
                    half = slice(0, DK) if kt % 2 == 0 else slice(DK, P)
                    tp = (0, 0) if kt % 2 == 0 else (DK, 0)
                    nc.tensor.matmul(
                        ps[:, j * QB : (j + 1) * QB],
                        kT[h][half, kt * P : (kt + 1) * P],
                        qT[h][half, qs],
                        start=True,
                        stop=True,
                        tile_position=tp,
                    )
                if (g * kgroup) % EXP_TILES == 0:
                    eb = exp_pool.tile([P, EXP_TILES * QB], BF16, tag="e", name="expT")
                    exp_bufs.append(eb)
                off = (g * kgroup) % EXP_TILES
                nc.scalar.activation(
                    out=exp_bufs[-1][:, off * QB : (off + gsz) * QB],
                    in_=ps[:, 0 : gsz * QB],
                    func=mybir.ActivationFunctionType.Exp,
                    scale=0.125,
                )
            return exp_bufs

        def emit_av_norm(qb, h, exp_bufs):
            av = ps_mm.tile([P, QB], F32, tag="mm", name="ps_av_t")
            for kt in range(KT_N):
                eb = exp_bufs[kt // EXP_TILES]
                off = kt % EXP_TILES
                nc.tensor.matmul(
                    av[0 : DK + 1],
                    v[h][:, kt, 0 : DK + 1],
                    eb[:, off * QB : (off + 1) * QB],
                    start=(kt == 0),
                    stop=(kt == KT_N - 1),
                )
            rt = small.tile([P, QB], F32R, tag="rt", name="recip_t")
            with nc.allow_low_precision(reason="f32r recip feeds f32r matmul"):
                nc.vector.reciprocal(out=rt[DK : DK + 1], in_=av[DK : DK + 1])
            # broadcast the reciprocal row across partitions 0..63 with a
            # K=1 PE matmul against a ones column (row group 64)
            rb = ps_mm.tile([P, QB], F32, tag="mmo", name="ps_rb")
            nc.tensor.matmul(
                rb[0:DK],
                ones1[DK : DK + 1, :],
                rt[DK : DK + 1, :],
                start=True,
                stop=True,
                tile_position=(DK, 0),
            )
            rbs = small.tile([DK, QB], F32, tag="rbs", name="rb_sb")
            nc.vector.tensor_copy(out=rbs, in_=rb[0:DK])
            o = small.tile([DK, QB], F32R, tag=f"oT{h}", name="oT_t")
            nc.vector.tensor_mul(out=o, in0=av[0:DK], in1=rbs)
            oT[(qb, h)] = o

        def emit_outproj(qb):
            qs = slice(qb * QB, (qb + 1) * QB)
            ysb = ysb_pool.tile([P, DT, QB], F32, tag="y", name="y_t")
            for dt_i in range(DT):
                pso = ps_mm.tile([P, QB], F32, tag="mmo", name="ps_o")
                for h in range(HPC):
                    nc.tensor.matmul(
                        pso,
                        wo_sb[:, h, dt_i * P : (dt_i + 1) * P],
                        oT[(qb, h)],
                        start=(h == 0),
                        stop=(h == HPC - 1),
                    )
                nc.vector.tensor_copy(out=ysb[:, dt_i, :], in_=pso)
            nc.sync.dma_start(
                out=yT.rearrange("(t p) n -> p t n", p=P)[:, :, qs], in_=ysb
            )

        pend = None  # (qb, h, exp_bufs) of the previous unit
        for qb, h in units:
            ebs = emit_scores_exp(qb, h)
            if pend is not None:
                pqb, ph, pebs = pend
                emit_av_norm(pqb, ph, pebs)
                if ph == HPC - 1:
                    emit_outproj(pqb)
            pend = (qb, h, ebs)
        pqb, ph, pebs = pend
        emit_av_norm(pqb, ph, pebs)
        emit_outproj(pqb)

    nc.compile()
    return nc


def shard_inputs(x, encoding, w_q, b_q, w_k, b_k, w_v, b_v, w_o, b_o):
    """Full inputs -> list of 8 per-core input dicts (numpy, contiguous)."""
    N = x.shape[1]
    xT_full = np.ascontiguousarray(np.asarray(x, np.float32)[0].T)      # [D, N]
    encT = np.ascontiguousarray(np.asarray(encoding, np.float32)[0].T)  # [D, N]
    w_q, w_k, w_v, w_o = (np.asarray(a, np.float32) for a in (w_q, w_k, w_v, w_o))
    b_q, b_k = np.asarray(b_q, np.float32), np.asarray(b_k, np.float32)
    in_maps = []
    for core in range(N_CORES):
        p = core // 2
        hsel = slice(HPC * p * DK, HPC * (p + 1) * DK)
        qsel = slice(0, N // 2) if core % 2 == 0 else slice(N // 2, N)
        in_maps.append(
            {
                "xT": np.ascontiguousarray(xT_full[:, qsel]),
                "encT": encT,
                "wkv": np.ascontiguousarray(
                    np.concatenate([w_k[:, hsel], w_v[:, hsel]], axis=1)
                ),
                "wq": np.ascontiguousarray(w_q[:, hsel]),
                "wo": np.ascontiguousarray(w_o[hsel, :]),
                "bq": np.ascontiguousarray(b_q[hsel].reshape(-1, 1)),
                "bk": np.ascontiguousarray(b_k[hsel].reshape(-1, 1)),
                "onesk": np.ones((1, N // P), ml_dtypes.bfloat16),
                "ones64": np.ones((1, DK), np.float32),
            }
        )
    return in_maps


def combine_outputs(results, b_v, w_o, b_o, N, dtype):
    """Per-core yT partials -> full [1, N, D] output (host-side biases)."""
    half = N // 2
    y = np.zeros((N, D), np.float32)
    for core, res in enumerate(results):
        yT_part = np.asarray(res["yT"], np.float32)
        if core % 2 == 0:
            y[:half] += yT_part.T
        else:
            y[half:] += yT_part.T
    y += np.asarray(b_v, np.float32) @ np.asarray(w_o, np.float32) + np.asarray(
        b_o, np.float32
    )
    return np.ascontiguousarray(y[None]).astype(dtype)


_PROGRAM_CACHE = {}


def _get_program():
    key = "main"
    if key not in _PROGRAM_CACHE:
        _PROGRAM_CACHE[key] = build_program()
    return _PROGRAM_CACHE[key]


def kernel(x, encoding, w_q, b_q, w_k, b_k, w_v, b_v, w_o, b_o):
    nc = _get_program()
    in_maps = shard_inputs(x, encoding, w_q, b_q, w_k, b_k, w_v, b_v, w_o, b_o)
    res = run_bass_kernel_spmd(nc, in_maps, core_ids=list(range(N_CORES)))
    return combine_outputs(
        res.results, b_v, w_o, b_o, np.asarray(x).shape[1], np.asarray(x).dtype
    )
